# revision 1
# baseline (speedup 1.0000x reference)
"""Trainium2 Bass kernel for nn_AdvLossForTarget_max.

reference:
    prob = softmax(input, axis=1); p = prob[:, -1]
    w = where(p == 1.0, 1 - 1e-6, 1.0)
    loss = mean(log(1 - p * w))        # scalar, float32

`target` is unused by the reference; it is accepted and ignored here.

Strategy: data-parallel over 8 NeuronCores (32768 rows each). The host
casts the input to fp16 (costs ~4e-6 relative on the final scalar while
halving HBM traffic). Each core streams its (32768, 1001) fp16 shard in
2 MB DMAs of 8 row-blocks of 128x1001; rows are padded to 1004 columns in
SBUF (pads zeroed once; 1004 keeps every access stride off 32-byte
multiples — 32B-multiple strides trip a chip-wide ~1.2x clock-down).
Per block the scalar engine does one strided in-place exp over the
8x1001 data columns (~6.9 us; exp on ACT is the pacing cost at ~220 us).
The vector engine reduces each row: two in-place fp16 fold-adds
(1004->502->250, one 3D-AP op each covering all 8 sub-rows), then a
252-wide accumulate per row into f32 S, plus one strided copy of
exp(x[:, -1]) into E. A small tail computes p = E/S, applies the p==1
mask, takes log(1 - p*w) and reduces to a [128,1] partial per core; the
host sums 8*128 partials and divides by B.
"""

import numpy as np

import concourse.bass as bass
from concourse import mybir
from concourse.bass_utils import run_bass_kernel_spmd

B, C = 262144, 1001
N_CORES = 8
ROWS_PER_CORE = B // N_CORES  # 32768
P = 128
CP = 1004  # padded row length in SBUF (stride 2008 B, not a 32B multiple)
SUB = 8  # 128-row blocks per DMA (2 MB fp16 each)
N_DMA = ROWS_PER_CORE // (P * SUB)  # 32
N_SUB = N_DMA * SUB  # 256
NBUF = 6
EPS = 1e-6
FP32 = mybir.dt.float32
FP16 = mybir.dt.float16
AF = mybir.ActivationFunctionType
ALU = mybir.AluOpType

BIGP = SUB * CP


def build_nc():
    nc = bass.Bass("TRN2", target_bir_lowering=False, debug=False)
    x = nc.declare_dram_parameter("input", [ROWS_PER_CORE, C], FP16, isOutput=False)
    out = nc.declare_dram_parameter("out", [P, 1], FP32, isOutput=True)
    # big-tile I: partition p holds rows I*1024 + 8p + t (t = 0..7), each a
    # contiguous 2 KB run in HBM. Row->partition mapping is irrelevant: the
    # loss is a global mean over rows.
    x_t = x.rearrange("(n p t) m -> n p t m", t=SUB, p=P)

    with (
        nc.sbuf_tensor([P, NBUF * BIGP], FP16) as tbuf,
        nc.sbuf_tensor([P, N_SUB], FP32) as S,
        nc.sbuf_tensor([P, N_SUB], FP16) as E,
        nc.sbuf_tensor([P, N_SUB], FP32) as Ef,
        nc.sbuf_tensor([P, N_SUB], FP32) as Pv,
        nc.sbuf_tensor([P, N_SUB], FP32) as M,
        nc.sbuf_tensor([P, N_SUB], FP32) as L,
        nc.sbuf_tensor([P, 252], FP16) as dump,
        nc.sbuf_tensor([P, 1], FP32) as acc,
        nc.semaphore("dma_sem") as dma_sem,
        nc.semaphore("act_sem") as act_sem,
        nc.semaphore("dve_sem") as dve_sem,
        nc.Block() as block,
    ):
        t4 = tbuf.rearrange("p (s t m) -> p s t m", t=SUB, m=CP)  # [P,NBUF,SUB,CP]
        NSPLIT = 2  # leading tiles split into sub-DMAs so ACT starts sooner
        LAST = N_DMA - 1  # last tile: ACT-fused accum, no DVE folds
        KA = (N_DMA - 1) * SUB  # columns final before the last tile

        # DMA completion targets: tiles 0..NSPLIT-1 contribute SUB sub-DMAs
        # each; tile i >= NSPLIT is one DMA.
        def dma_done(i):
            return 16 * (NSPLIT * SUB + (i - NSPLIT) + 1)

        @block.sync
        def _(sp):
            for i in range(NSPLIT):
                for j in range(SUB):
                    sp.dma_start(
                        out=t4[:, i, j, 0:C], in_=x_t[i, :, j, :]
                    ).then_inc(dma_sem, 16)
            for i in range(NSPLIT, N_DMA):
                if i >= NBUF:
                    sp.wait_ge(dve_sem, i - NBUF + 1)
                slot = i % NBUF
                sp.dma_start(
                    out=t4[:, slot, :, 0:C], in_=x_t[i, :, :, :]
                ).then_inc(dma_sem, 16)
            sp.wait_ge(dve_sem, N_DMA + 2)
            sp.dma_start(out=out[:, :], in_=acc[:, :]).then_inc(dma_sem, 16)
            sp.wait_ge(dma_sem, 16 * (NSPLIT * SUB + N_DMA - NSPLIT + 1))

        @block.scalar
        def _(act):
            for i in range(NSPLIT):
                for j in range(SUB):
                    act.wait_ge(dma_sem, 16 * (i * SUB + j + 1))
                    ins = nc.scalar.activation(
                        t4[:, i, j, 0:C], t4[:, i, j, 0:C], AF.Exp
                    )
                ins.then_inc(act_sem, 1)
            for i in range(NSPLIT, N_DMA - 1):
                act.wait_ge(dma_sem, dma_done(i))
                slot = i % NBUF
                nc.scalar.activation(
                    t4[:, slot, :, 0:C], t4[:, slot, :, 0:C], AF.Exp
                ).then_inc(act_sem, 1)
            # last tile as two halves so DVE's folds pipeline with the
            # final exps instead of trailing them
            act.wait_ge(dma_sem, dma_done(LAST))
            slot = LAST % NBUF
            H = SUB // 2
            nc.scalar.activation(
                t4[:, slot, 0:H, 0:C], t4[:, slot, 0:H, 0:C], AF.Exp
            ).then_inc(act_sem, 1)
            nc.scalar.activation(
                t4[:, slot, H:SUB, 0:C], t4[:, slot, H:SUB, 0:C], AF.Exp
            ).then_inc(act_sem, 1)
            act.wait_ge(dve_sem, N_DMA + 1)
            nc.scalar.activation(
                L[:, :], Pv[:, :], AF.Ln, bias=1.0, scale=-1.0
            ).then_inc(act_sem, 1)

        @block.vector
        def _(dve):
            # zero pad columns per slot (slot-indexed so no 32B-multiple
            # stride appears in any AP); pads only feed DVE's later folds
            for s in range(NBUF):
                nc.vector.memset(t4[:, s, :, C:CP], 0)
            for i in range(N_DMA - 1):
                dve.wait_ge(act_sem, i + 1)
                slot = i % NBUF
                g = t4[:, slot, :, :]
                # exp(x[:, -1]) for the 8 sub-rows, one strided copy
                nc.vector.tensor_copy(
                    E[:, i * SUB : (i + 1) * SUB], g[:, :, C - 1 : C]
                )
                # folds: 1004 -> 502 -> 250 (+2 raw), pads contribute 0
                nc.vector.tensor_add(g[:, :, 0:502], g[:, :, 0:502], g[:, :, 502:1004])
                nc.vector.tensor_add(g[:, :, 0:250], g[:, :, 0:250], g[:, :, 252:502])
                for j in range(SUB):
                    k = i * SUB + j
                    ts = nc.vector.tensor_scalar(
                        dump[:, :],
                        g[:, j, 0:252],
                        1.0,
                        0.0,
                        op0=ALU.mult,
                        op1=ALU.add,
                        accum_out=S[:, k : k + 1],
                    )
                ts.then_inc(dve_sem, 1)
            # overlap tail math for the first half (final after tile 15);
            # chunks stay >=128 wide: narrow reciprocal ops return garbage
            KB = N_SUB - 128
            nc.vector.tensor_copy(Ef[:, 0:KB], E[:, 0:KB])
            nc.vector.reciprocal(M[:, 0:KB], S[:, 0:KB])
            nc.vector.tensor_mul(Pv[:, 0:KB], Ef[:, 0:KB], M[:, 0:KB])
            nc.vector.tensor_scalar(
                M[:, 0:KB], Pv[:, 0:KB], 1.0, None, op0=ALU.is_equal
            )
            nc.vector.scalar_tensor_tensor(
                Pv[:, 0:KB], M[:, 0:KB], -EPS, Pv[:, 0:KB], op0=ALU.mult, op1=ALU.add
            )
            # last tile, two halves pipelined against ACT's final exps
            slot = LAST % NBUF
            H = SUB // 2
            for h in range(2):
                dve.wait_ge(act_sem, N_DMA + h)
                gh = t4[:, slot, h * H : (h + 1) * H, :]
                lo = KA + h * H
                nc.vector.tensor_copy(E[:, lo : lo + H], gh[:, :, C - 1 : C])
                nc.vector.tensor_add(
                    gh[:, :, 0:502], gh[:, :, 0:502], gh[:, :, 502:1004]
                )
                nc.vector.tensor_add(
                    gh[:, :, 0:250], gh[:, :, 0:250], gh[:, :, 252:502]
                )
                for j in range(H):
                    k = lo + j
                    nc.vector.tensor_scalar(
                        dump[:, :],
                        gh[:, j, 0:252],
                        1.0,
                        0.0,
                        op0=ALU.mult,
                        op1=ALU.add,
                        accum_out=S[:, k : k + 1],
                    )
            nc.vector.tensor_copy(Ef[:, KB:N_SUB], E[:, KB:N_SUB]).then_inc(dve_sem, 1)
            nc.vector.reciprocal(M[:, KB:N_SUB], S[:, KB:N_SUB])
            nc.vector.tensor_mul(Pv[:, KB:N_SUB], Ef[:, KB:N_SUB], M[:, KB:N_SUB])
            nc.vector.tensor_scalar(
                M[:, KB:N_SUB], Pv[:, KB:N_SUB], 1.0, None, op0=ALU.is_equal
            )
            nc.vector.scalar_tensor_tensor(
                Pv[:, KB:N_SUB],
                M[:, KB:N_SUB],
                -EPS,
                Pv[:, KB:N_SUB],
                op0=ALU.mult,
                op1=ALU.add,
            ).then_inc(dve_sem, 1)
            dve.wait_ge(act_sem, N_DMA + 2)  # Ln done (halves used +1/+2)
            nc.vector.reduce_sum(
                acc[:, :], L[:, :], axis=mybir.AxisListType.X
            ).then_inc(dve_sem, 1)

    return nc


def _shard16(x):
    x16 = np.asarray(x, dtype=np.float16)
    return [
        np.ascontiguousarray(x16[i * ROWS_PER_CORE : (i + 1) * ROWS_PER_CORE])
        for i in range(N_CORES)
    ]


def kernel(**inputs):
    x = inputs["input"]
    assert tuple(x.shape) == (B, C)
    nc = build_nc()
    in_maps = [{"input": s} for s in _shard16(x)]
    res = run_bass_kernel_spmd(nc, in_maps, list(range(N_CORES))).results
    total = 0.0
    for r in res:
        total += float(np.sum(r["out"].astype(np.float64)))
    return np.array(total / B, dtype=np.float32)

